# revision 37
# baseline (speedup 1.0000x reference)
"""Causal self-attention (B=4, S=2048, D=1024, H=16) on 8 NeuronCores.

Sharding: core c handles batch b = c//2 and head-group g = c%2 (8 heads).
Each core computes qkv for its head group, causal attention for its 8 heads,
and a partial projection (its 512 rows of W_proj). Host sums the two partial
outputs per batch and adds b_proj.

Device layout notes:
 - x is passed transposed (xT [D, S]) and bf16; qT/kT are computed in
   [qkv_col, token] layout so the scores matmul needs no transposes:
   scoresT[k_tok, q_tok] = kT_tile.T @ qT  (lhsT = kT, contraction = head dim).
 - softmax runs on scoresT: exp on ScalarE (scale=1/8 folded in); the causal
   mask is applied in PSUM by adding a 0/-1e9 triangular tile to the diagonal
   128-wide band via an identity matmul (PE), and the masked prefix of each
   k-tile row is simply never computed or accumulated.
 - denominators come from a ones-column appended to v (v_aug [k,65]); the
   ctx matmul then yields [ctx(64 rows); sums(1 row)] per q block.
 - normalization: reciprocal of the sums row, partition-broadcast on GpSimd,
   one fused multiply+cast on VectorE.
"""

import numpy as np
import ml_dtypes

import concourse.bacc as bacc
import concourse.tile as tile
from concourse import mybir
from concourse.bass_utils import run_bass_kernel_spmd

BF16 = mybir.dt.bfloat16
F32 = mybir.dt.float32
EXP = mybir.ActivationFunctionType.Exp

B = 4
S = 2048  # tokens per batch
D = 1024
HG = 8    # heads per core
HD = 64
GC = HG * HD  # 512 qkv columns per core per q/k/v
N_CORES = 8
SCALE = 0.125  # 1/sqrt(64)


def _body(nc, xT, wq, wk, wv, wp, bqkv, tri, ident, outT, tc, layout="fill", use_bias=True):
    _const_cm = tc.tile_pool(name="const", bufs=1)
    const = _const_cm.__enter__()
    qT_sb = const.tile([128, 4, S], BF16)
    kT_sb = const.tile([128, 4, S], BF16)
    ctxT_sb = const.tile([128, 4, S], BF16)
    vaug_sb = const.tile([128, 16, HG, 65], BF16)
    wp_sb = const.tile([128, 4, D], BF16)
    tri_sb = const.tile([128, 128], BF16)
    ident_sb = const.tile([128, 128], BF16)
    b_sb = const.tile([1, 3 * GC], BF16)
    ones1 = const.tile([1, 512], BF16)

    nc.vector.memset(ones1[:], 1.0)
    nc.vector.memset(vaug_sb[:, :, :, 64:65], 1.0)
    nc.sync.dma_start(out=tri_sb[:], in_=tri.ap())
    nc.sync.dma_start(out=ident_sb[:], in_=ident.ap())
    nc.sync.dma_start(out=b_sb[:], in_=bqkv.ap())
    for ct in range(4):
        nc.sync.dma_start(out=wp_sb[:, ct, :], in_=wp.ap()[128 * ct:128 * (ct + 1), :])

    xT_sb = const.tile([128, 8, S], BF16)
    wq_sb = const.tile([128, 8, GC], BF16)
    wk_sb = const.tile([128, 8, GC], BF16)
    wv_sb = const.tile([128, 8, GC], BF16)
    # xT + wq stream first so the first qk matmuls can start ASAP
    for t in range(8):
        nc.sync.dma_start(out=xT_sb[:, t, :], in_=xT.ap()[128 * t:128 * (t + 1), :])
        nc.sync.dma_start(out=wq_sb[:, t, :], in_=wq.ap()[128 * t:128 * (t + 1), :])
    for t in range(8):
        nc.sync.dma_start(out=wk_sb[:, t, :], in_=wk.ap()[128 * t:128 * (t + 1), :])
    for t in range(8):
        nc.sync.dma_start(out=wv_sb[:, t, :], in_=wv.ap()[128 * t:128 * (t + 1), :])

    # One shared PSUM pool scheme across all phases so emission can pipeline:
    #   scp "sc": [128,1024] slots x2 (4 banks) - qkv psums, scores, proj
    #   cxp "cx": [65,512] slots x4 (4 banks)   - ctx accumulators
    _scp_cm = tc.tile_pool(name="scp", bufs=2, space="PSUM")
    scp = _scp_cm.__enter__()
    _cxp_cm = tc.tile_pool(name="cxp", bufs=4, space="PSUM")
    cxp = _cxp_cm.__enter__()
    _prp_cm = tc.tile_pool(name="prp", bufs=6)
    prp = _prp_cm.__enter__()
    _nrm_cm = tc.tile_pool(name="nrm", bufs=4)
    nrm = _nrm_cm.__enter__()

    def qk_group(c, qk, tb):
        w_sb, dst, boff = ((wq_sb, qT_sb, 0), (wk_sb, kT_sb, GC))[qk]
        ps = scp.tile([128, 512], F32, tag="sc", name=f"qk_{c}_{boff}_{tb}")
        for t in range(8):
            nc.tensor.matmul(
                ps[:],
                lhsT=w_sb[:, t, 128 * c:128 * (c + 1)],
                rhs=xT_sb[:, t, 512 * tb:512 * (tb + 1)],
                start=(t == 0), stop=(not use_bias and t == 7))
        if use_bias:
            nc.tensor.matmul(
                ps[:],
                lhsT=b_sb[0:1, boff + 128 * c: boff + 128 * (c + 1)],
                rhs=ones1[0:1, :],
                start=False, stop=True)
        nc.vector.tensor_copy(dst[:, c, 512 * tb:512 * (tb + 1)], ps[:])

    def v_tile(j):
        # v in natural [token, v_col] layout, + bias, scattered into v_aug
        psv = scp.tile([128, 512], F32, tag="sc", name=f"pv_{j}")
        for t in range(8):
            nc.tensor.matmul(
                psv[:],
                lhsT=xT_sb[:, t, 128 * j:128 * (j + 1)],
                rhs=wv_sb[:, t, :],
                start=(t == 0), stop=(not use_bias and t == 7))
        if use_bias:
            nc.tensor.matmul(
                psv[:],
                lhsT=ones1[0:1, 0:128],
                rhs=b_sb[0:1, 2 * GC:3 * GC],
                start=False, stop=True)
        nc.vector.tensor_copy(
            vaug_sb[:, j, :, 0:64],
            psv[:].rearrange("p (h c) -> p h c", h=HG))

    def normalize(h, qb, ctx_ps):
        o = 64 * (h % 2)
        c = h // 2
        rec = nrm.tile([1, 512], F32, tag="rec", name=f"rec_{h}_{qb}")
        nc.vector.reciprocal(rec[:], ctx_ps[qb][64:65, :])
        bc = nrm.tile([64, 512], F32, tag="bc", name=f"bc_{h}_{qb}")
        nc.gpsimd.partition_broadcast(bc[:], rec[:])
        if o == 0:
            nc.vector.tensor_mul(
                ctxT_sb[0:64, c, 512 * qb:512 * (qb + 1)],
                ctx_ps[qb][0:64, :], bc[:])
        else:
            stg = nrm.tile([64, 512], BF16, tag="stg", name=f"stg_{h}_{qb}")
            nc.vector.tensor_mul(stg[:], ctx_ps[qb][0:64, :], bc[:])
            nc.sync.dma_start(
                out=ctxT_sb[64:128, c, 512 * qb:512 * (qb + 1)], in_=stg[:])

    _ob3_cm = tc.tile_pool(name="ob3", bufs=4)
    ob3 = _ob3_cm.__enter__()

    def proj_group(m, tb):
        ps = scp.tile([128, 512], F32, tag="sc", name=f"p3_{m}_{tb}")
        for ct in range(4):
            nc.tensor.matmul(
                ps[:],
                lhsT=wp_sb[:, ct, 128 * m:128 * (m + 1)],
                rhs=ctxT_sb[:, ct, 512 * tb:512 * (tb + 1)],
                start=(ct == 0), stop=(ct == 3))
        ob = ob3.tile([128, 512], F32, tag="o3", name=f"ob_{m}_{tb}")
        # ACT is idle by the time the projection runs; keep DVE free
        nc.scalar.copy(ob[:], ps[:])
        nc.sync.dma_start(
            out=outT.ap()[128 * m:128 * (m + 1), 512 * tb:512 * (tb + 1)],
            in_=ob[:])

    def h7_filler(j):
        # tb-block tb of the projection becomes legal once head 7's q-block
        # tb is normalized at j = 4*tb + 3; emit 2 (m, tb) groups per j
        if j >= 4:
            idx = j - 4
            tb, pair = idx // 4, idx % 4
            proj_group(2 * pair, tb)
            proj_group(2 * pair + 1, tb)

    def head_block(h, filler=None):
        o = 64 * (h % 2)
        c = h // 2
        ctx_ps = [cxp.tile([65, 512], F32, tag="cx", name=f"cx_{h}_{qb}")
                  for qb in range(4)]
        for j in range(16):
            if filler is not None:
                filler(j)
            qbm, r = divmod(j, 4)
            width = S - 512 * qbm
            rel0 = 128 * r
            pT = prp.tile([128, S], BF16, tag="probs", name=f"pT_{h}_{j}")
            # scores chunks of <=1024 free, one exp per chunk; the causal mask
            # is applied in PSUM by adding tri_neg (0 / -1e9) to the diagonal
            # 128-wide band via an identity matmul, keeping the whole
            # scores->exp chain on PE->ACT only
            for ch0 in range(0, width, 1024):
                ch1 = min(ch0 + 1024, width)
                lo = max(ch0, rel0)
                if lo >= ch1:
                    continue
                ps = scp.tile([128, 1024], F32, tag="sc", name=f"sc_{h}_{j}_{ch0}")
                for qb in range(qbm + ch0 // 512, qbm + ch1 // 512):
                    rq0 = (qb - qbm) * 512
                    mlo = max(rq0, rel0)
                    diag = mlo == rel0 and ch0 == 0
                    nc.tensor.matmul(
                        ps[:, mlo - ch0: rq0 + 512 - ch0],
                        lhsT=kT_sb[o:o + 64, c, 128 * j:128 * (j + 1)],
                        rhs=qT_sb[o:o + 64, c,
                                  512 * qbm + mlo: 512 * qbm + rq0 + 512],
                        start=True, stop=not diag, skip_group_check=True)
                    if diag:
                        nc.tensor.matmul(
                            ps[:, rel0 - ch0: rel0 - ch0 + 128],
                            lhsT=ident_sb[:],
                            rhs=tri_sb[:],
                            start=False, stop=True, skip_group_check=True)
                nc.scalar.activation(
                    pT[:, lo:ch1], ps[:, lo - ch0:ch1 - ch0], EXP, scale=SCALE)
            # ctx accumulation (with sums in row 64); the diagonal block's
            # masked prefix [0, rel0) is never computed nor accumulated
            for qb in range(qbm, 4):
                lo = rel0 if qb == qbm else 0
                nc.tensor.matmul(
                    ctx_ps[qb][:, lo:512],
                    lhsT=vaug_sb[:, j, h, :],
                    rhs=pT[:, (qb - qbm) * 512 + lo: (qb - qbm + 1) * 512],
                    start=(j == 0), stop=(j == 4 * qb + 3))
            if r == 3:
                # qb = (j-3)//4 just received its last accumulation
                normalize(h, (j - 3) // 4, ctx_ps)

    def spread(groups):
        stride = max(1, 16 // max(1, len(groups)))
        def f(j):
            i = j // stride
            if j % stride == 0 and i < len(groups):
                groups[i]()
        return f

    qkg = [[(lambda c=c, qk=qk, tb=tb: qk_group(c, qk, tb))
            for qk in range(2) for tb in range(4)] for c in range(4)]
    if layout == "fill":
        # qk(0) upfront; v interleaved into h0 two iterations ahead of use;
        # qk(1..3) spread into h1..h5
        for g in qkg[0]:
            g()
        v_tile(0)
        v_tile(1)
        head_block(0, filler=lambda j: v_tile(j + 2) if j < 14 else None)
        head_block(1, filler=spread(qkg[1]))
        head_block(2, filler=spread(qkg[2][:4]))
        head_block(3, filler=spread(qkg[2][4:]))
        head_block(4, filler=spread(qkg[3][:4]))
        head_block(5, filler=spread(qkg[3][4:]))
        head_block(6)
        head_block(7, filler=h7_filler)
    elif layout == "seq":
        # all qkv upfront, then pure attention heads
        for c in range(4):
            for g in qkg[c]:
                g()
        for j in range(16):
            v_tile(j)
        for h in range(HG - 1):
            head_block(h)
        head_block(7, filler=h7_filler)
    elif layout == "block":
        # qkv blocks between head pairs
        for g in qkg[0]:
            g()
        for j in range(16):
            v_tile(j)
        for c in range(4):
            if c:
                for g in qkg[c]:
                    g()
            head_block(2 * c)
            head_block(2 * c + 1, filler=h7_filler if c == 3 else None)
    else:
        raise ValueError(layout)


    for pair in range(4):
        proj_group(2 * pair, 3)
        proj_group(2 * pair + 1, 3)

    _ob3_cm.__exit__(None, None, None)
    _nrm_cm.__exit__(None, None, None)
    _prp_cm.__exit__(None, None, None)
    _cxp_cm.__exit__(None, None, None)
    _scp_cm.__exit__(None, None, None)
    _const_cm.__exit__(None, None, None)


CW = 1024  # scores/exp psum chunk width; banks: scp 2x2 + cx 3 + tx 1 = 8


def _body_v2(nc, xT, wq, wk, wv, wp, tri, ident, outT, tc, dbg=None):
    """v2: ctx accumulated as [q-partition, d-free] (streams 65 cols per
    (j, q-chunk) instead of re-streaming the q range), denominators fused as
    vaug's ones column -> per-partition tensor_scalar normalize (no Pool
    broadcast), PE-transpose of normalized head pairs back into ctxT layout,
    proj accumulates from ctxT and DMAs straight out of PSUM."""
    _const_cm = tc.tile_pool(name="const", bufs=1)
    const = _const_cm.__enter__()
    xT_sb = const.tile([128, 8, S], BF16)
    wq_sb = const.tile([128, 8, GC], BF16)
    wk_sb = const.tile([128, 8, GC], BF16)
    wv_sb = const.tile([128, 8, GC], BF16)
    wp_sb = const.tile([128, 4, D], BF16)
    qT_sb = const.tile([128, 4, S], BF16)
    kT_sb = const.tile([128, 4, S], BF16)
    ctxT_sb = const.tile([128, 4, S], BF16)
    vaug_sb = const.tile([128, 16, HG, 65], BF16)
    stage_sb = const.tile([128, 16, 128], BF16)  # shared across pairs (WAR)
    tri_sb = const.tile([128, 128], BF16)
    ident_sb = const.tile([128, 128], BF16)

    nc.vector.memset(vaug_sb[:, :, :, 64:65], 1.0)
    # Descriptor gen is ~625ns serial per dma_start on the sync queue, so
    # few+ordered DMAs: ident (scores diag needs it), wq as ONE strided DMA,
    # x tb0 halves (first qk groups), wk, wv, the rest of x, wp, tri.
    nc.sync.dma_start(out=ident_sb[:], in_=ident.ap())
    nc.sync.dma_start(
        out=wq_sb[:], in_=wq.ap().rearrange("(t p) c -> p t c", p=128))
    for t in range(8):
        nc.sync.dma_start(
            out=xT_sb[:, t, 0:512], in_=xT.ap()[128 * t:128 * (t + 1), 0:512])
    nc.sync.dma_start(
        out=wk_sb[:], in_=wk.ap().rearrange("(t p) c -> p t c", p=128))
    nc.sync.dma_start(
        out=wv_sb[:], in_=wv.ap().rearrange("(t p) c -> p t c", p=128))
    nc.sync.dma_start(out=tri_sb[:], in_=tri.ap())
    for t in range(8):
        nc.sync.dma_start(
            out=xT_sb[:, t, 512:2048],
            in_=xT.ap()[128 * t:128 * (t + 1), 512:2048])
    nc.sync.dma_start(
        out=wp_sb[:], in_=wp.ap().rearrange("(t p) c -> p t c", p=128))

    # PSUM banks: sc 2x2 + cx 2x1 + tx 1 + pj 1 = 8.  A PSUM bank supports
    # only ONE active accumulation group at a time, so ctx accumulates
    # chunk-major (all jj of a chunk back-to-back).
    _scp_cm = tc.tile_pool(name="scp", bufs=2, space="PSUM")
    scp = _scp_cm.__enter__()
    _cxp_cm = tc.tile_pool(name="cxp", bufs=2, space="PSUM")
    cxp = _cxp_cm.__enter__()
    _txp_cm = tc.tile_pool(name="txp", bufs=1, space="PSUM")
    txp = _txp_cm.__enter__()
    _pjp_cm = tc.tile_pool(name="pjp", bufs=1, space="PSUM")
    pjp = _pjp_cm.__enter__()
    _prp_cm = tc.tile_pool(name="prp", bufs=10)
    prp = _prp_cm.__enter__()
    _prn_cm = tc.tile_pool(name="prn", bufs=9)
    prn = _prn_cm.__enter__()
    _rcp_cm = tc.tile_pool(name="rcp", bufs=4)
    rcp = _rcp_cm.__enter__()
    _obp_cm = tc.tile_pool(name="obp", bufs=4)
    obp = _obp_cm.__enter__()

    def qk_group(c, qk, tb, act_copy=False):
        w_sb, dst = ((wq_sb, qT_sb), (wk_sb, kT_sb))[qk]
        ps = scp.tile([128, 512], F32, tag="sc", name=f"qk_{c}_{qk}_{tb}")
        for t in range(8):
            nc.tensor.matmul(
                ps[:],
                lhsT=w_sb[:, t, 128 * c:128 * (c + 1)],
                rhs=xT_sb[:, t, 512 * tb:512 * (tb + 1)],
                start=(t == 0), stop=(t == 7))
        eng = nc.scalar.copy if act_copy else nc.vector.tensor_copy
        eng(dst[:, c, 512 * tb:512 * (tb + 1)], ps[:])

    def v_tile(j, act_copy=False):
        psv = scp.tile([128, 512], F32, tag="sc", name=f"pv_{j}")
        for t in range(8):
            nc.tensor.matmul(
                psv[:],
                lhsT=xT_sb[:, t, 128 * j:128 * (j + 1)],
                rhs=wv_sb[:, t, :],
                start=(t == 0), stop=(t == 7))
        eng = nc.scalar.copy if act_copy else nc.vector.tensor_copy
        eng(vaug_sb[:, j, :, 0:64],
            psv[:].rearrange("p (h c) -> p h c", h=HG))

    def scores_exp(h, j, pT, off_pt, chunk_filler=None):
        c, o = h // 2, 64 * (h % 2)
        rel0 = 128 * j
        ci = 0
        for c0 in range(0, S, CW):
            c1 = min(c0 + CW, S)
            lo = max(c0, rel0)
            if lo >= c1:
                continue
            if chunk_filler is not None:
                chunk_filler(ci)
            ci += 1
            ps = scp.tile([128, CW], F32, tag="sc", name=f"sc_{h}_{j}_{c0}")
            # scores segments of <=512 moving cols; matmul writes must not
            # cross a psum bank boundary, so break at 512-aligned tile offs;
            # a segment's group closes immediately unless the diag band
            # (tri add) overlaps it
            a = lo
            while a < c1:
                b = min(c0 + ((a - c0) // 512 + 1) * 512, c1)
                s0, s1 = max(a, rel0), min(b, rel0 + 128)
                diag = s0 < s1
                nc.tensor.matmul(
                    ps[:, a - c0:b - c0],
                    lhsT=kT_sb[o:o + 64, c, 128 * j:128 * (j + 1)],
                    rhs=qT_sb[o:o + 64, c, a:b],
                    start=True, stop=not diag, skip_group_check=True)
                if diag:
                    nc.tensor.matmul(
                        ps[:, s0 - c0:s1 - c0],
                        lhsT=ident_sb[:],
                        rhs=tri_sb[:, s0 - rel0:s1 - rel0],
                        start=False, stop=True, skip_group_check=True)
                a = b
            nc.scalar.activation(
                pT[:, lo - off_pt:c1 - off_pt], ps[:, lo - c0:c1 - c0],
                EXP, scale=SCALE)

    tx_tiles = {}

    def head_block_v2(h, filler=None, chunk_fillers=None):
        c, o = h // 2, 64 * (h % 2)
        fine = (h == 7)
        if c not in tx_tiles:
            # one psum bank per pair: eight 128-wide transpose sub-slots,
            # rotated so consecutive groups never alias
            tx_tiles[c] = txp.tile([128, 1024], BF16, tag="tx", name=f"tx_{c}")
        pTs = [None] * 16
        offs = [0 if j < 8 else 1024 for j in range(16)]
        cxt = [None]

        def norm_pair(cx, ms):
            # denominators sit in col 64 of each chunk (ones column of vaug)
            rec = rcp.tile([128, len(ms)], F32, tag="rec",
                           name=f"rec_{h}_{ms[0]}")
            nc.vector.reciprocal(rec[:], cx[:, 0:len(ms), 64:65])
            for i, m in enumerate(ms):
                nc.vector.tensor_scalar_mul(
                    stage_sb[:, m, o:o + 64], cx[:, i, 0:64],
                    rec[:, i:i + 1])

        def transpose_chunks(m0, n):
            tx = tx_tiles[c]
            s = 128 * (2 * ((m0 // 2) % 4) if n > 1 else (m0 % 4) * 2)
            for i in range(n):
                nc.tensor.transpose(
                    tx[:, s + 128 * i:s + 128 * (i + 1)],
                    stage_sb[:, m0 + i, :], ident_sb[:])
            nc.vector.tensor_copy(
                ctxT_sb[:, c, 128 * m0:128 * (m0 + n)],
                tx[:, s:s + 128 * n])

        for slot in range(17):
            if filler is not None:
                filler(slot)
            if slot < 16:
                j = slot
                pool, tag, w = (prp, "probs", S) if j < 8 else (prn, "prn", S // 2)
                pTs[j] = pool.tile([128, w], BF16, tag=tag, name=f"pT_{h}_{j}")
                cf = chunk_fillers.get(slot) if chunk_fillers else None
                scores_exp(h, j, pTs[j], offs[j], chunk_filler=cf)
            if slot >= 1:
                # chunk-major ctx: all jj of chunk m back-to-back (one
                # active psum accumulation group per bank at a time)
                m = slot - 1
                sub = m % 2
                if sub == 0:
                    cxt[0] = cxp.tile([128, 2, 65], F32, tag="cx",
                                      name=f"cx_{h}_{m}")
                cx = cxt[0]
                for jj in range(0, m + 1):
                    nc.tensor.matmul(
                        cx[:, sub, :],
                        lhsT=pTs[jj][:, 128 * m - offs[jj]:
                                      128 * (m + 1) - offs[jj]],
                        rhs=vaug_sb[:, jj, h, 0:65],
                        start=(jj == 0), stop=(jj == m),
                        skip_group_check=True)
                if fine and m >= 12:
                    # per-chunk norm/transpose so tail proj can start early
                    nc_rec = rcp.tile([128, 1], F32, tag="rec",
                                      name=f"rec_{h}_{m}")
                    nc.vector.reciprocal(nc_rec[:], cx[:, sub, 64:65])
                    nc.vector.tensor_scalar_mul(
                        stage_sb[:, m, o:o + 64], cx[:, sub, 0:64],
                        nc_rec[:, 0:1])
                    if o == 64:
                        transpose_chunks(m, 1)
                elif sub == 1:
                    norm_pair(cx, [m - 1, m])
                    if o == 64:
                        transpose_chunks(m - 1, 2)

    def proj_group(m_, q0, w):
        pj = pjp.tile([128, 512], F32, tag="pj", name=f"pj_{m_}_{q0}")
        for ct in range(4):
            nc.tensor.matmul(
                pj[:, 0:w],
                lhsT=wp_sb[:, ct, 128 * m_:128 * (m_ + 1)],
                rhs=ctxT_sb[:, ct, q0:q0 + w],
                start=(ct == 0), stop=(ct == 3))
        ob = obp.tile([128, 512], F32, tag="ob", name=f"ob_{m_}_{q0}")
        nc.vector.tensor_copy(ob[:, 0:w], pj[:, 0:w])
        nc.sync.dma_start(
            out=outT.ap()[128 * m_:128 * (m_ + 1), q0:q0 + w],
            in_=ob[:, 0:w])

    # ---- emission ----
    # startup: just enough of qk c0 for h0 scores j0 chunk0 (q cols 0:1024,
    # k block 0); the rest interleaves into h0's chunk/slot fillers
    qk_group(0, 0, 0, act_copy=True)
    qk_group(0, 0, 1, act_copy=True)
    qk_group(0, 1, 0, act_copy=True)

    def dict_filler(d):
        def f(slot):
            for u in d.get(slot, ()):
                u()
        return f

    qkg = {c: [(lambda c=c, qk=qk, tb=tb: qk_group(c, qk, tb))
               for qk in range(2) for tb in range(4)] for c in range(1, 4)}

    # h0: remaining q c0 between scores j0 chunks; v tiles one slot ahead of
    # their ctx-block use; k c0 tb1..3 ahead of scores j4/j8/j12
    h0_slots = {
        0: [lambda: v_tile(0, act_copy=True)],
        1: [lambda: v_tile(1), lambda: qk_group(0, 1, 1)],
        2: [lambda: v_tile(2)],
        3: [lambda: v_tile(3), lambda: qk_group(0, 1, 2)],
        4: [lambda: v_tile(4)],
        5: [lambda: v_tile(5), lambda: qk_group(0, 1, 3)],
    }
    for j in range(6, 16):
        h0_slots[j] = [lambda j=j: v_tile(j)]
    h0_slots[14].append(qkg[1][0])
    h0_slots[15] = h0_slots.get(15, []) + [qkg[1][1]]
    head_block_v2(
        0, filler=dict_filler(h0_slots),
        chunk_fillers={0: lambda ci: (qk_group(0, 0, 2), qk_group(0, 0, 3))
                       if ci == 1 else None})
    # h1: c1 remaining 6
    head_block_v2(1, filler=dict_filler(
        {0: [qkg[1][2]], 2: [qkg[1][3]], 4: [qkg[1][4]], 6: [qkg[1][5]],
         8: [qkg[1][6]], 10: [qkg[1][7]]}))
    # h2/h3: c2
    head_block_v2(2, filler=dict_filler(
        {1: [qkg[2][0]], 5: [qkg[2][1]], 9: [qkg[2][2]], 13: [qkg[2][3]]}))
    head_block_v2(3, filler=dict_filler(
        {1: [qkg[2][4]], 5: [qkg[2][5]], 9: [qkg[2][6]], 13: [qkg[2][7]]}))
    # h4/h5/h6: c3 (k tb1..3 inside h6, just ahead of use)
    head_block_v2(4, filler=dict_filler({3: [qkg[3][0]], 9: [qkg[3][1]]}))
    head_block_v2(5, filler=dict_filler(
        {2: [qkg[3][2]], 7: [qkg[3][3]], 12: [qkg[3][4]]}))

    proj_units = []
    for tb in range(3):
        for m_ in range(8):
            proj_units.append((lambda m_=m_, tb=tb: proj_group(m_, 512 * tb, 512)))

    def h6_filler(slot):
        if slot in (0, 2, 4):
            qkg[3][5 + (slot // 2)]()

    head_block_v2(6, filler=h6_filler)

    def h7_filler(slot):
        # tb-block tb of proj legal once h7's chunk 4tb+3 is transposed at
        # slot 4tb+5; 2 groups per slot starting at slot 5
        if slot >= 5:
            i = 2 * (slot - 5)
            for k in (i, i + 1):
                if k < len(proj_units):
                    proj_units[k]()
        if slot == 16:
            # chunks 12..14 ready; proj cols 1536:1920
            for m_ in range(8):
                proj_group(m_, 1536, 384)

    head_block_v2(7, filler=h7_filler)
    # tail: last 128 cols after h7 chunk 15
    for m_ in range(8):
        proj_group(m_, 1920, 128)

    if dbg is not None:
        nc.sync.dma_start(out=dbg["stage"].ap(),
                          in_=stage_sb[:].rearrange("p m q -> p (m q)"))
        nc.sync.dma_start(out=dbg["ctxT"].ap(),
                          in_=ctxT_sb[:].rearrange("p c s -> p (c s)"))
        nc.sync.dma_start(out=dbg["qT"].ap(),
                          in_=qT_sb[:].rearrange("p c s -> p (c s)"))
        nc.sync.dma_start(out=dbg["kT"].ap(),
                          in_=kT_sb[:].rearrange("p c s -> p (c s)"))
        nc.sync.dma_start(out=dbg["vaug"].ap(),
                          in_=vaug_sb[:].rearrange("p j h c -> p (j h c)"))

    _obp_cm.__exit__(None, None, None)
    _rcp_cm.__exit__(None, None, None)
    _prn_cm.__exit__(None, None, None)
    _prp_cm.__exit__(None, None, None)
    _pjp_cm.__exit__(None, None, None)
    _txp_cm.__exit__(None, None, None)
    _cxp_cm.__exit__(None, None, None)
    _scp_cm.__exit__(None, None, None)
    _const_cm.__exit__(None, None, None)


_CACHED = {}
_DBG = False


def _build(reps=1, layout="fill", use_bias=True):
    key = (reps, layout, use_bias)
    if key in _CACHED:
        return _CACHED[key]
    nc = bacc.Bacc()
    xT = nc.dram_tensor("xT", [D, S], BF16, kind="ExternalInput")
    wq = nc.dram_tensor("wq", [D, GC], BF16, kind="ExternalInput")
    wk = nc.dram_tensor("wk", [D, GC], BF16, kind="ExternalInput")
    wv = nc.dram_tensor("wv", [D, GC], BF16, kind="ExternalInput")
    wp = nc.dram_tensor("wp", [GC, D], BF16, kind="ExternalInput")
    bqkv = nc.dram_tensor("bqkv", [1, 3 * GC], BF16, kind="ExternalInput")
    tri = nc.dram_tensor("tri", [128, 128], BF16, kind="ExternalInput")
    ident = nc.dram_tensor("ident", [128, 128], BF16, kind="ExternalInput")
    outT = nc.dram_tensor("outT", [D, S], F32, kind="ExternalOutput")
    dbg = None
    if layout == "v2" and reps == 1 and _DBG:
        dbg = {
            "stage": nc.dram_tensor("dbg_stage", [128, 16 * 128], BF16,
                                    kind="ExternalOutput"),
            "ctxT": nc.dram_tensor("dbg_ctxT", [128, 4 * S], BF16,
                                   kind="ExternalOutput"),
            "qT": nc.dram_tensor("dbg_qT", [128, 4 * S], BF16,
                                 kind="ExternalOutput"),
            "kT": nc.dram_tensor("dbg_kT", [128, 4 * S], BF16,
                                 kind="ExternalOutput"),
            "vaug": nc.dram_tensor("dbg_vaug", [128, 16 * 8 * 65], BF16,
                                   kind="ExternalOutput"),
        }
    with tile.TileContext(nc) as tc:
        for _ in range(reps):
            if layout == "v2":
                _body_v2(nc, xT, wq, wk, wv, wp, tri, ident, outT, tc, dbg=dbg)
            else:
                _body(nc, xT, wq, wk, wv, wp, bqkv, tri, ident, outT, tc, layout=layout, use_bias=use_bias)
    nc.compile()
    _CACHED[key] = nc
    return nc


def make_in_maps(x, W_attn, b_attn, W_proj):
    bf = ml_dtypes.bfloat16
    tri_np = np.where(np.arange(128)[None, :] >= np.arange(128)[:, None],
                      np.float32(0.0), np.float32(-1e9)).astype(bf)
    ident_np = np.eye(128, dtype=np.float32).astype(bf)
    in_maps = []
    for core in range(N_CORES):
        b, g = divmod(core, 2)
        cols = slice(GC * g, GC * (g + 1))
        in_maps.append({
            "xT": np.ascontiguousarray(x[b].T).astype(bf),
            "wq": np.ascontiguousarray(W_attn[:, cols]).astype(bf),
            "wk": np.ascontiguousarray(W_attn[:, D:][:, cols]).astype(bf),
            "wv": np.ascontiguousarray(W_attn[:, 2 * D:][:, cols]).astype(bf),
            "wp": np.ascontiguousarray(W_proj[cols, :]).astype(bf),
            "bqkv": np.concatenate(
                [b_attn[cols], b_attn[D:][cols], b_attn[2 * D:][cols]]
            ).reshape(1, 3 * GC).astype(bf),
            "tri": tri_np,
            "ident": ident_np,
        })
    return in_maps


def kernel(x, W_attn, b_attn, W_proj, b_proj, _run_kwargs=None):
    x = np.asarray(x)
    W_attn = np.asarray(W_attn)
    b_attn = np.asarray(b_attn)
    W_proj = np.asarray(W_proj)
    b_proj = np.asarray(b_proj)

    use_bias = bool(np.any(b_attn))
    nc = _build(layout="fill" if use_bias else "v2", use_bias=use_bias)
    in_maps = make_in_maps(x, W_attn, b_attn, W_proj)

    res = run_bass_kernel_spmd(
        nc, in_maps, core_ids=list(range(N_CORES)), **(_run_kwargs or {}))

    out = np.empty((B, S, D), np.float32)
    for b in range(B):
        acc = res.results[2 * b]["outT"] + res.results[2 * b + 1]["outT"]
        out[b] = acc.T + b_proj[None, :].astype(np.float32)
    if _run_kwargs:
        kernel.last_results = res
    return out

